# revision 12
# baseline (speedup 1.0000x reference)
import os, sys
sys.path.insert(0, "/opt/trn_rl_repo")
import numpy as np

N_CORES = 8
B_FULL = 8192
S = B_FULL // N_CORES        # 1024 samples per core
SUB = 128                    # samples per conv sub-chunk
BLK = 256                    # samples per stage-B block
VGROUPS = [(0, 4), (4, 4), (8, 4)]  # (V0, Vn) groups covering V=0..11

_cache = {}


def _compose_conv9(c1w, c1b, c2w, c2b):
    # compose two VALID cross-correlations into one 9x9 cross-correlation
    c1w = c1w.astype(np.float64); c2w = c2w.astype(np.float64)
    w9 = np.zeros((20, 3, 9, 9))
    for ey in range(5):
        for ex in range(5):
            # w2 tap (ey,ex) acts on conv1 output shifted by (ey,ex)
            w9[:, :, ey:ey + 5, ex:ex + 5] += np.einsum(
                'om,mcab->ocab', c2w[:, :, ey, ex], c1w)
    bA = np.einsum('omyx,m->o', c2w, c1b.astype(np.float64)) + c2b.astype(np.float64)
    return w9, bA


def _stageB_np(z, c3w, c3b, pw, pb, f1w, f1b, f2w, f2b, f3w, f3b):
    # z: [n, 20, 12, 12] float64 -> [n, 10]; mirrors reference post-pool ops
    n = z.shape[0]
    y = np.zeros((n, 29, 10, 10))
    for dy in range(3):
        for dx in range(3):
            y += np.einsum('oc,ncij->noij', c3w[:, :, dy, dx],
                           z[:, :, dy:dy + 10, dx:dx + 10])
    y += c3b[None, :, None, None]
    idx1 = 2 * np.arange(14) + 1
    idx2 = idx1 + 1
    p = (pw[:, 0][None, :, None, None] * y[:, 0:1]
         + pw[:, 1][None, :, None, None] * y[:, idx1]
         + pw[:, 2][None, :, None, None] * y[:, idx2]
         + pb[None, :, None, None])
    f = p.reshape(n, -1)
    f = f @ f1w.T + f1b
    f = f @ f2w.T + f2b
    return f @ f3w.T + f3b


def _prep_weights(inp):
    w9, bA = _compose_conv9(inp['conv1_w'], inp['conv1_b'],
                            inp['conv2_w'], inp['conv2_b'])
    args64 = [inp[k].astype(np.float64) for k in
              ('conv3_w', 'conv3_b', 'p_w', 'p_b', 'fc1_w', 'fc1_b',
               'fc2_w', 'fc2_b', 'fc3_w', 'fc3_b')]
    cF = _stageB_np(np.zeros((1, 20, 12, 12)), *args64)[0]          # [10]
    I = np.eye(2880).reshape(2880, 20, 12, 12)
    MF = (_stageB_np(I, *args64) - cF[None, :]).T                    # [10, 2880]
    bA_field = np.broadcast_to(bA[:, None, None], (20, 12, 12)).reshape(1, 20, 12, 12)
    b_final = _stageB_np(bA_field, *args64)[0]                       # [10]

    # conv lhsT tiles: W9L[b*9+tx] : [96, 128]
    w9l = np.zeros((4, 9, 3, 32, 128), dtype=np.float32)
    for ty in range(9):
        for b in range(4):
            for u in range(3):
                for par in range(2):
                    y_in = 6 * b + 2 * u + par + ty
                    cols = par * 64 + np.arange(20) * 3 + u
                    w9l[b, :, :, y_in, cols] = w9[:, :, ty, :].transpose(0, 2, 1)
    w9l = w9l.reshape(36, 96, 128)
    # stage-B tiles: WB[V*4+b] : [60, 10]
    MFz = MF.reshape(10, 20, 12, 12)
    wb = np.zeros((12, 4, 20, 3, 10), dtype=np.float32)
    for b in range(4):
        for u in range(3):
            # feat Y = 3b+u, X = V
            wb[:, b, :, u, :] = MFz[:, :, 3 * b + u, :].transpose(2, 1, 0)
    wb = wb.reshape(48, 60, 10)
    bf = b_final.astype(np.float32).reshape(10, 1)
    return w9l, wb, bf


def _build_nc():
    import concourse.bass as bass
    import concourse.tile as tile
    from concourse import bacc, mybir

    f32 = mybir.dt.float32
    f32r = mybir.dt.float32r

    nc = bacc.Bacc("TRN2", target_bir_lowering=False, debug=False, num_devices=1)
    x_d = nc.dram_tensor("x", [S, 3, 32, 32], f32r, kind="ExternalInput").ap()
    w9_d = nc.dram_tensor("w9l", [36, 96, 128], f32r, kind="ExternalInput").ap()
    wb_d = nc.dram_tensor("wb", [48, 60, 10], f32r, kind="ExternalInput").ap()
    bf_d = nc.dram_tensor("bf", [10, 1], f32, kind="ExternalInput").ap()
    out_d = nc.dram_tensor("out", [10, S], f32, kind="ExternalOutput").ap()

    with tile.TileContext(nc) as tc:
        with (
            tc.tile_pool(name="wpool", bufs=1) as wpool,
            tc.tile_pool(name="xpool", bufs=3) as xpool,
            tc.tile_pool(name="pooled", bufs=2) as plpool,
            tc.tile_pool(name="outp", bufs=2) as outp,
            tc.tile_pool(name="psc", bufs=6, space="PSUM") as psc,
            tc.tile_pool(name="ps3", bufs=2, space="PSUM") as ps3,
        ):
            w9sb = wpool.tile([96, 36 * 128], f32r)
            nc.sync.dma_start(w9sb[:], w9_d.rearrange("t p m -> p t m"))
            wbsb = wpool.tile([60, 48 * 10], f32r)
            nc.sync.dma_start(wbsb[:], wb_d.rearrange("t p m -> p t m"))
            bfsb = wpool.tile([10, 1], f32)
            nc.sync.dma_start(bfsb[:], bf_d[:])

            n_blk = S // BLK
            n_sub = BLK // SUB
            rep = int(os.environ.get('KERNEL_REPEAT', '1'))
            for blk_r in range(n_blk * rep):
                blk = blk_r % n_blk
                pooled = {}
                for (gi, (V0, Vn)) in enumerate(VGROUPS):
                    for b in range(4):
                        pooled[(b, gi)] = plpool.tile([60, BLK * Vn], f32r,
                                                      name=f"pl{b}_{gi}_{blk_r}",
                                                      tag=f"pl{b}_{gi}")
                for sub in range(n_sub):
                    s0 = blk * BLK + sub * SUB
                    X1 = xpool.tile([96, SUB * 32], f32r)
                    nc.sync.dma_start(
                        X1[:],
                        x_d[s0:s0 + SUB].rearrange("s c y x -> (c y) s x"))
                    X1v = X1[:].rearrange("p (s x) -> p s x", x=32)
                    for b in range(4):
                        for (gi, (V0, Vn)) in enumerate(VGROUPS):
                            ps_par = []
                            for par in range(2):
                                ps = psc.tile([128, SUB * Vn], f32, tag="conv")
                                for tx in range(9):
                                    x_in0 = 2 * V0 + par + tx
                                    rhs = X1v[:, :, x_in0:x_in0 + 2 * Vn - 1:2]
                                    nc.tensor.matmul(
                                        ps[:],
                                        w9sb[:, (b * 9 + tx) * 128:(b * 9 + tx) * 128 + 128],
                                        rhs,
                                        start=(tx == 0), stop=(tx == 8))
                                ps_par.append(ps)
                            pE, pO = ps_par
                            dst = pooled[(b, gi)][:, sub * SUB * Vn:(sub + 1) * SUB * Vn]
                            nc.vector.tensor_copy(dst, pE[0:60, :])
                            nc.vector.tensor_max(dst, dst, pE[64:124, :])
                            nc.vector.tensor_max(dst, dst, pO[0:60, :])
                            nc.vector.tensor_max(dst, dst, pO[64:124, :])
                # stage B for this block
                p3 = ps3.tile([10, BLK], f32)
                first = True
                for (gi, (V0, Vn)) in enumerate(VGROUPS):
                    for v in range(Vn):
                        V = V0 + v
                        for b in range(4):
                            t = V * 4 + b
                            pl = pooled[(b, gi)][:].rearrange(
                                "p (s v) -> p v s", v=Vn)
                            last = (gi == len(VGROUPS) - 1 and v == Vn - 1 and b == 3)
                            nc.tensor.matmul(
                                p3[:], wbsb[:, t * 10:t * 10 + 10], pl[:, v, :],
                                start=first, stop=last)
                            first = False
                ob = outp.tile([10, BLK], f32)
                nc.vector.tensor_scalar_add(ob[:], p3[:], bfsb[:])
                nc.sync.dma_start(out_d[:, blk * BLK:(blk + 1) * BLK], ob[:])

    nc.compile()
    return nc


def kernel(**inputs):
    import concourse.bass_utils as bass_utils

    if 'nc' not in _cache:
        _cache['nc'] = _build_nc()
    nc = _cache['nc']

    w9l, wb, bf = _prep_weights(inputs)
    x = np.ascontiguousarray(inputs['x'], dtype=np.float32)
    in_maps = []
    for c in range(N_CORES):
        in_maps.append({
            'x': x[c * S:(c + 1) * S],
            'w9l': w9l, 'wb': wb, 'bf': bf,
        })
    trace = bool(int(os.environ.get('KERNEL_TRACE', '0')))
    res = bass_utils.run_bass_kernel_spmd(nc, in_maps,
                                          core_ids=list(range(N_CORES)),
                                          trace=trace)
    _cache['last_result'] = res
    out = np.concatenate([res.results[c]['out'].T for c in range(N_CORES)], axis=0)
    return out.astype(np.float32)
